# revision 3
# baseline (speedup 1.0000x reference)
"""Trainium2 Bass kernel for nn_BipartiteGATLayer (8-core SPMD).

Math
----
reference computes, with h = X @ W, sl = h @ a_left, sr = h @ a_right:
    scores[i, j] = sl[i] + sr[j]
    attn = softmax_j(where(adj > 0, scores, -9e15))
    out = elu(attn @ h)
The row-constant sl[i] cancels inside the row softmax, and the -9e15
entries underflow exp to exactly 0 in fp32, so with e = exp(sr + C)
(any constant shift C):
    out[i, :] = elu( (Af[i, :] @ (e * h)) / (Af[i, :] @ e) ),  Af = (adj > 0)
i.e. one [N, N] x [N, 65] matmul against the 0/1 adjacency, where the
65th column of the right operand is e itself (giving the denominator).

Distribution (row-shard per the hint)
-------------------------------------
Each of the 8 cores owns N/8 = 1024 query rows of adjacency/X.  Each
core computes h/e/G for its own 1024 j-rows from its X slice, then an
AllGather shares G = [e*h | e] (zero-row-padded, see below) so every
core holds the full right operand, and each core computes its 1024
output rows locally.

Adjacency transpose trick
-------------------------
The PE contracts over the SBUF partition dim, so adjacency tiles are
needed j-on-partitions, i.e. transposed vs. the row-major DRAM layout.
The xbar DMA-transpose handles 16-bit elements only; adjacency is
int32 0/1, i.e. int16 pairs [v, 0].  Transposing the int16 *view*
puts values on even partitions and exact zeros on odd partitions; the
zero partitions contribute nothing to the matmul contraction, and G is
laid out with matching zero rows interleaved (done via the DRAM
AllGather layout, keeping all access patterns dense and regular).
The int16 0/1 values are bitcast to fp16 (0x0001 = 2^-24 subnormal);
the uniform 2^-24 scale cancels in the numerator/denominator ratio.
If the PE flushes subnormal inputs to zero, set USE_SUBNORMAL = False
to fall back to an explicit DVE int16->fp16 cast pass.
"""

import math
from functools import lru_cache

import numpy as np

import concourse.bacc as bacc
import concourse.mybir as mybir
import concourse.tile as tile
from concourse.bass_utils import run_bass_kernel_spmd
from concourse.masks import make_identity

# Problem shape (hardcoded per harness contract).
N = 8192
IN_F = 512
OUT_F = 64
NCORES = 8

P = 128
GD = OUT_F + 1          # G columns: 64 feature cols + 1 denominator col
E_SHIFT = -10.0 * math.log(2.0)  # exp prescale 2^-10 keeps e*h inside fp16 range

USE_SUBNORMAL = True    # bitcast int16 0/1 as fp16 subnormals (no cast pass)

F32 = mybir.dt.float32
F16 = mybir.dt.float16
I16 = mybir.dt.int16
I32 = mybir.dt.int32

AluOp = mybir.AluOpType
ActFn = mybir.ActivationFunctionType


def _build_kernel(tc, a_ap, x_ap, w_ap, av_ap, out_ap, n_total, rows,
                  use_subnormal):
    nc = tc.nc
    rt = rows // P               # row-tiles (i) per core
    dc_n = IN_F // P             # contraction chunks for X @ W
    npair = rt // 2              # row-tile pairs (N=256 moving operand)
    nq = 4                       # xbar chunks per row-tile
    hwq = (2 * n_total) // nq    # halfwords per xbar chunk
    bpq = hwq // P               # 128-halfword blocks per chunk
    nb = nq * bpq                # j-blocks total (64 real j each)

    a16 = a_ap.bitcast(I16)      # [rows, 2*n_total] halfword view

    with (
        tc.tile_pool(name="const", bufs=1) as cpool,
        tc.tile_pool(name="xh", bufs=2) as xpool,
        tc.tile_pool(name="at", bufs=4) as atpool,
        tc.tile_pool(name="work", bufs=2) as wpool,
        tc.tile_pool(name="pacc", bufs=2, space="PSUM") as ppool,
        tc.tile_pool(name="ppre", bufs=2, space="PSUM") as ppoolA,
        tc.tile_pool(name="dram", bufs=1, space="DRAM") as dpool,
    ):
        # ---- constants ----
        id64 = cpool.tile([OUT_F, OUT_F], F32, tag="id64")
        make_identity(nc, id64[:])
        id128 = cpool.tile([P, P], F32, tag="id128")
        make_identity(nc, id128[:])

        w_sb = cpool.tile([P, dc_n, OUT_F], F32, tag="w")
        nc.sync.dma_start(out=w_sb[:], in_=w_ap.rearrange("(c p) f -> p c f", p=P))

        ar1 = cpool.tile([1, OUT_F], F32, tag="ar1")
        nc.sync.dma_start(out=ar1[:],
                          in_=av_ap[OUT_F:2 * OUT_F, :].rearrange("f o -> o f"))
        arb = cpool.tile([P, OUT_F], F32, tag="arb")
        nc.gpsimd.partition_broadcast(arb[:], ar1[:])

        # ---- X^T via PE transposes, then h / e / G for this core's rows ----
        xT = cpool.tile([P, dc_n, rows], F32, tag="xT")
        for it in range(rt):
            x_sb = xpool.tile([P, IN_F], F32, tag="x")
            nc.sync.dma_start(out=x_sb[:], in_=x_ap[it * P:(it + 1) * P, :])
            for dc in range(dc_n):
                pt = ppoolA.tile([P, P], F32, tag="ptr")
                nc.tensor.transpose(pt[:], x_sb[:, dc * P:(dc + 1) * P], id128[:])
                nc.vector.tensor_copy(xT[:, dc, it * P:(it + 1) * P], pt[:])

        # G staged with interleaved zero rows: g_sb[p, jt, 0, :] = [e*h | e],
        # g_sb[p, jt, 1, :] = 0, so the gathered DRAM image is row 2j+parity.
        g_sb = cpool.tile([P, rt, 2, GD], F16, tag="g")
        nc.gpsimd.memset(g_sb[:], 0.0)
        for jt in range(rt):
            ph = ppoolA.tile([P, OUT_F], F32, tag="ph")
            for dc in range(dc_n):
                nc.tensor.matmul(
                    ph[:],
                    xT[:, dc, jt * P:(jt + 1) * P],
                    w_sb[:, dc, :],
                    start=(dc == 0),
                    stop=(dc == dc_n - 1),
                )
            h_sb = xpool.tile([P, OUT_F], F32, tag="h")
            nc.vector.tensor_copy(h_sb[:], ph[:])
            tmp = xpool.tile([P, OUT_F], F32, tag="tmp")
            nc.vector.tensor_tensor(tmp[:], h_sb[:], arb[:], op=AluOp.mult)
            sr = xpool.tile([P, 1], F32, tag="sr")
            nc.vector.reduce_sum(sr[:], tmp[:], axis=mybir.AxisListType.X)
            nc.vector.tensor_scalar_add(sr[:], sr[:], E_SHIFT)
            ee = xpool.tile([P, 1], F32, tag="ee")
            nc.scalar.activation(ee[:], sr[:], ActFn.Exp)
            nc.vector.tensor_scalar_mul(g_sb[:, jt, 0, 0:OUT_F], h_sb[:], ee[:])
            nc.vector.tensor_copy(g_sb[:, jt, 0, OUT_F:GD], ee[:])

        # ---- AllGather G (zero-interleaved rows) ----
        g_loc = dpool.tile([2 * rows, GD], F16, tag="gloc")
        nc.sync.dma_start(
            out=g_loc.rearrange("(jt p two) f -> p jt two f", p=P, two=2),
            in_=g_sb[:],
        )
        g_all = dpool.tile([2 * n_total, GD], F16, tag="gall")
        nc.gpsimd.collective_compute(
            "AllGather",
            AluOp.bypass,
            replica_groups=[list(range(NCORES))],
            ins=[g_loc.opt()],
            outs=[g_all.opt()],
        )

        # g_int[p, b, f] = g_all[128*b + p, f]; even p = G row, odd p = 0.
        g_int = cpool.tile([P, nb, GD], F16, tag="gint")
        nc.sync.dma_start(out=g_int[:],
                          in_=g_all.rearrange("(b p) f -> p b f", p=P))

        # ---- main loop: stream transposed adjacency, accumulate A @ G ----
        for tp in range(npair):
            t0 = 2 * tp
            pp = ppool.tile([GD, 2, P], F32, tag="acc")
            for q in range(nq):
                at = atpool.tile([P, bpq, 2, P], I16, tag="at")
                for ti in range(2):
                    t = t0 + ti
                    nc.sync.dma_start(
                        out=at[:, :, ti, :],
                        in_=a16[t * P:(t + 1) * P, q * hwq:(q + 1) * hwq],
                        transpose=True,
                    )
                if use_subnormal:
                    rhs = at.bitcast(F16)
                else:
                    rhs = atpool.tile([P, bpq, 2, P], F16, tag="atf")
                    nc.vector.tensor_copy(rhs[:], at[:])
                for b in range(bpq):
                    B = q * bpq + b
                    nc.tensor.matmul(
                        pp[:],
                        g_int[:, B, :],
                        rhs[:, b, :, :],
                        start=(q == 0 and b == 0),
                        stop=(q == nq - 1 and b == bpq - 1),
                    )

            # out rows = num / den, then ELU, then transpose back to natural.
            den = pp[OUT_F:GD, :, :]
            rec = wpool.tile([1, 2, P], F32, tag="rec")
            nc.vector.reciprocal(rec[:], den)
            # Newton refinement: rec *= (2 - den * rec)
            nt = wpool.tile([1, 2, P], F32, tag="nt")
            nc.vector.tensor_tensor(nt[:], den, rec[:], op=AluOp.mult)
            nc.vector.tensor_scalar(nt[:], nt[:], -1.0, 2.0,
                                    op0=AluOp.mult, op1=AluOp.add)
            nc.vector.tensor_tensor(rec[:], rec[:], nt[:], op=AluOp.mult)

            rb = wpool.tile([OUT_F, 2, P], F32, tag="rb")
            nc.gpsimd.partition_broadcast(rb[:], rec[:])
            oT = wpool.tile([OUT_F, 2, P], F32, tag="oT")
            nc.vector.tensor_tensor(oT[:], pp[0:OUT_F, :, :], rb[:], op=AluOp.mult)
            em = wpool.tile([OUT_F, 2, P], F32, tag="em")
            nc.scalar.activation(em[:], oT[:], ActFn.Exp)
            nc.vector.tensor_scalar_add(em[:], em[:], -1.0)
            mk = wpool.tile([OUT_F, 2, P], mybir.dt.uint8, tag="mk")
            nc.vector.tensor_scalar(mk[:], oT[:], 0.0, None, op0=AluOp.is_gt)
            nc.vector.copy_predicated(em[:], mk[:], oT[:])
            for ti in range(2):
                po = ppool.tile([P, OUT_F], F32, tag="po")
                nc.tensor.transpose(po[:], em[:, ti, :], id64[:])
                so = wpool.tile([P, OUT_F], F32, tag="so")
                nc.vector.tensor_copy(so[:], po[:])
                t = t0 + ti
                nc.sync.dma_start(out=out_ap[t * P:(t + 1) * P, :], in_=so[:])


@lru_cache(maxsize=2)
def build_program(n_total=N, use_subnormal=USE_SUBNORMAL):
    rows = n_total // NCORES
    nc = bacc.Bacc("TRN2", target_bir_lowering=False, debug=False,
                   num_devices=NCORES)
    a_in = nc.dram_tensor("a_slice", [rows, n_total], I32, kind="ExternalInput")
    x_in = nc.dram_tensor("x_slice", [rows, IN_F], F32, kind="ExternalInput")
    w_in = nc.dram_tensor("w", [IN_F, OUT_F], F32, kind="ExternalInput")
    av_in = nc.dram_tensor("av", [2 * OUT_F, 1], F32, kind="ExternalInput")
    out = nc.dram_tensor("out", [rows, OUT_F], F32, kind="ExternalOutput")

    with tile.TileContext(nc) as tc:
        _build_kernel(tc, a_in.ap(), x_in.ap(), w_in.ap(), av_in.ap(),
                      out.ap(), n_total, rows, use_subnormal)
    nc.compile()
    return nc


def make_in_maps(node_features, adjacency_matrix, weight_matrix,
                 attention_vector, n_total=N):
    rows = n_total // NCORES
    x = np.ascontiguousarray(np.asarray(node_features, dtype=np.float32))
    a = np.asarray(adjacency_matrix, dtype=np.int32)
    w = np.ascontiguousarray(np.asarray(weight_matrix, dtype=np.float32))
    av = np.ascontiguousarray(np.asarray(attention_vector, dtype=np.float32))
    return [
        {
            "a_slice": np.ascontiguousarray(a[c * rows:(c + 1) * rows, :]),
            "x_slice": np.ascontiguousarray(x[c * rows:(c + 1) * rows, :]),
            "w": w,
            "av": av,
        }
        for c in range(NCORES)
    ]


def kernel(node_features, adjacency_matrix, weight_matrix, attention_vector):
    nc = build_program()
    in_maps = make_in_maps(node_features, adjacency_matrix, weight_matrix,
                           attention_vector)
    res = run_bass_kernel_spmd(nc, in_maps, list(range(NCORES)))
    out = np.concatenate([r["out"] for r in res.results], axis=0)
    return out
